# revision 29
# baseline (speedup 1.0000x reference)
"""Trainium2 Bass kernel for nn_ActorMultiHead (moe_routing).

Strategy
--------
The reference runs every role head on every token (dense form of a masked
dispatch) and then selects the row matching the token's role; tokens whose
role >= NUM_ROLES contribute exactly 0.  We implement the sparse dispatch:

  * Host: flatten [B, A] tokens, drop role>=2 tokens (their output is 0),
    sort the rest by role, pack them into NT-token single-role tiles, and
    assign whole tiles to cores so every core serves exactly one role.
    The one-hot input contribution then folds into the layer-0 bias and the
    per-role heads become per-core weight *data* — the SPMD graph is
    role-agnostic.  NT=456 so 6 tiles x 8 cores exactly covers the ~21.8k
    active tokens (24 tiles per role -> 4 cores per role).
  * Device (per core, 3 pairs of tiles x NT tokens), mixed precision:
        layer 0 (obs->H)      : f32r matmul, contraction 128; both relu
                                paths store 1*h so blocks alternate engines
        layers 1,2 (H->H)     : fp8e4m3 DoubleRow matmuls (contraction 256)
        head 1 (H->H2)        : fp8 DoubleRow
        head 2 (H2->act)      : fp8 DoubleRow, M padded 8->16 (col 8 is the
                                const lane)
        log-prob tail         : lp = sum_k w_k (a_k - mean_k)^2 + c expanded
                                as W_red^T mean^2 + 1^T (mean * apre) with
                                apre = -2*w*a precomputed on host; the
                                constant ca = sum(w*a^2)+c rides in apre
                                row 8 against a tanh(bias=20)=1.0 lane.
                                The tanh/square/apre-mul chain is emitted at
                                the pair's end (runs during the next pair's
                                body); only the two small reduce matmuls +
                                copy + store are deferred to the next pair's
                                L2/H1 boundary, so the PE never waits.
    fp8 scaling: every fp8-layer PSUM uniformly holds P*u (P=16).  DR-layer
    ReLU blocks split between ScalarE (stores alpha*h, alpha=4) and VectorE
    (stores P*h); the per-block scale folds into the NEXT layer's fp8
    weight quantization (c_k = P / s_block(k)).  Layer-0 stores 1*h on both
    engines so each block's two tiles use both engines concurrently.
  * Host: scatter per-tile outputs back to original token positions.

For timing, a variant graph wraps the whole per-core compute in a
constant-bound For_i loop (set `kernel.nrep = R`), so a test harness can
measure HW time as (wall(nrep=R) - wall(nrep=1)) / (R-1).
"""

import math

import numpy as np
import ml_dtypes

# -- problem constants (from the problem statement, hardcoded) ---------------
B, A = 2048, 16
OBS_DIM, HIDDEN, ACTION_DIM = 128, 1024, 8
NUM_ROLES = 2
AGENT_ID_DIM = NUM_ROLES
H2 = HIDDEN // 2
LOG_2PI = math.log(2.0 * math.pi)

N_CORES = 8
NT = 456          # tokens per tile (matmul moving free dim)
CT = 6            # tiles per core (fixed compile-time shape)
C = NT * CT       # tokens per core per batch
KH = HIDDEN // 128    # 8 feature blocks of hidden
KZ = H2 // 128        # 4 feature blocks of the head hidden
KKH = KH // 2         # 4 DoubleRow k-pair tiles over hidden
KKZ = KZ // 2         # 2 DoubleRow k-pair tiles over head hidden
M2 = 16               # head-2 output padded to 16 (col 8 = const lane)

P_SC = 16.0       # uniform fp8-layer PSUM scale
ALPHA = 4.0       # ScalarE-block stored-activation scale (DR layers)

# per-block engine maps for DR layers: True -> ScalarE, False -> VectorE
MAP_L1 = (True, True, True, True, True, False, False, False)
MAP_L2 = (True, True, True, True, True, False, False, False)
MAP_H1 = (True, True, False, False)

F8 = ml_dtypes.float8_e4m3

_GRAPHS = {}  # repeats -> compiled graph, built once per process

TPART = 9  # tail partitions (rows 0..7 = actions, row 8 = const lane)


def _build_graph(repeats=1):
    import concourse.bass as bass
    import concourse.tile as tile
    from concourse import bacc, mybir

    f32 = mybir.dt.float32
    f32r = mybir.dt.float32r
    fp8 = mybir.dt.float8e4
    Act = mybir.ActivationFunctionType
    Alu = mybir.AluOpType
    DR = mybir.MatmulPerfMode.DoubleRow
    SWI = mybir.MatmulPerfMode.DoubleRowSwInterleave

    nc = bacc.Bacc(None, target_bir_lowering=False)

    xT = nc.declare_dram_parameter("xT", [CT, 128, NT], f32r, isOutput=False)
    w0 = nc.declare_dram_parameter("w0", [128, HIDDEN], f32r, isOutput=False)
    bias0 = nc.declare_dram_parameter("bias0", [128, KH], f32, isOutput=False)
    w1 = nc.declare_dram_parameter("w1", [128, KKH, KH, 256], fp8, isOutput=False)
    bias1 = nc.declare_dram_parameter("bias1", [128, KH], f32, isOutput=False)
    w2 = nc.declare_dram_parameter("w2", [128, KKH, KH, 256], fp8, isOutput=False)
    bias2 = nc.declare_dram_parameter("bias2", [128, KH], f32, isOutput=False)
    hw1 = nc.declare_dram_parameter("hw1", [128, KKH, KZ, 256], fp8, isOutput=False)
    hbias1 = nc.declare_dram_parameter("hbias1", [128, KZ], f32, isOutput=False)
    hw2 = nc.declare_dram_parameter("hw2", [128, KKZ, 2, M2], fp8, isOutput=False)
    hb2c = nc.declare_dram_parameter("hb2c", [TPART, 1], f32, isOutput=False)
    wredW = nc.declare_dram_parameter("wredW", [TPART, 1], f32r, isOutput=False)
    onesW = nc.declare_dram_parameter("onesW", [TPART, 1], f32r, isOutput=False)
    apre = nc.declare_dram_parameter("apre", [CT, TPART, NT], f32r, isOutput=False)
    out = nc.declare_dram_parameter("out", [CT, NT], f32, isOutput=True)

    with tile.TileContext(nc) as tc:
        with (
            tc.tile_pool(name="consts", bufs=1) as consts,
            tc.tile_pool(name="acts", bufs=2) as acts,
            tc.tile_pool(name="small", bufs=2) as small,
            tc.tile_pool(name="psum", bufs=6, space="PSUM") as psum,
            tc.tile_pool(name="psmall", bufs=2, space="PSUM") as psmall,
        ):
            # resident inputs, DMA'd roughly in the order compute needs them
            xt_sb = []
            xt0 = consts.tile([128, NT], f32r, name="xt0", uniquify=True)
            nc.sync.dma_start(xt0[:], xT[0])
            xt1 = consts.tile([128, NT], f32r, name="xt1", uniquify=True)
            nc.sync.dma_start(xt1[:], xT[1])
            xt_sb += [xt0, xt1]
            w0_sb = consts.tile([128, HIDDEN], f32r)
            nc.sync.dma_start(w0_sb[:], w0[:])
            b0_sb = consts.tile([128, KH], f32)
            nc.sync.dma_start(b0_sb[:], bias0[:])
            for t in range(2, CT):
                xt = consts.tile([128, NT], f32r, name=f"xt{t}", uniquify=True)
                nc.sync.dma_start(xt[:], xT[t])
                xt_sb.append(xt)
            w1_sb = consts.tile([128, KKH, KH, 256], fp8)
            nc.sync.dma_start(w1_sb[:], w1[:])
            b1_sb = consts.tile([128, KH], f32)
            nc.sync.dma_start(b1_sb[:], bias1[:])
            w2_sb = consts.tile([128, KKH, KH, 256], fp8)
            nc.sync.dma_start(w2_sb[:], w2[:])
            b2_sb = consts.tile([128, KH], f32)
            nc.sync.dma_start(b2_sb[:], bias2[:])
            hw1_sb = consts.tile([128, KKH, KZ, 256], fp8)
            nc.sync.dma_start(hw1_sb[:], hw1[:])
            hb1_sb = consts.tile([128, KZ], f32)
            nc.sync.dma_start(hb1_sb[:], hbias1[:])
            hw2_sb = consts.tile([128, KKZ, 2, M2], fp8)
            nc.sync.dma_start(hw2_sb[:], hw2[:])
            hb2c_sb = consts.tile([TPART, 1], f32)
            nc.sync.dma_start(hb2c_sb[:], hb2c[:])
            wredW_sb = consts.tile([TPART, 1], f32r)
            nc.sync.dma_start(wredW_sb[:], wredW[:])
            onesW_sb = consts.tile([TPART, 1], f32r)
            nc.sync.dma_start(onesW_sb[:], onesW[:])
            apre_sb = []
            for t in range(CT):
                ap_t = consts.tile([TPART, NT], f32r, name=f"apre{t}",
                                   uniquify=True)
                nc.sync.dma_start(ap_t[:], apre[t])
                apre_sb.append(ap_t)

            from contextlib import nullcontext

            def relu_l0(dest, m, ps, scalar_engine):
                # both engines store 1*h for layer 0
                dst = dest[:, m // 2, m % 2, :]
                bcol = b0_sb[:, m : m + 1]
                if scalar_engine:
                    nc.scalar.activation(dst, ps[:], Act.Relu, bias=bcol,
                                         scale=1.0)
                else:
                    nc.vector.tensor_scalar(dst, ps[:], bcol, 0.0,
                                            Alu.add, Alu.max)

            def emit_relu(dest, m, ps, bias_sb, emap):
                dst = dest[:, m // 2, m % 2, :]
                bcol = bias_sb[:, m : m + 1]
                if emap[m]:
                    nc.scalar.activation(dst, ps[:], Act.Relu, bias=bcol,
                                         scale=ALPHA / P_SC)
                else:
                    nc.vector.tensor_scalar(dst, ps[:], bcol, 0.0,
                                            Alu.add, Alu.max)

            def emit_dr_group(w_sb, src, ps, m, nkk):
                for kk in range(nkk):
                    nc.tensor.matmul(
                        ps[:], w_sb[:, kk, m, :], src[:, kk, :, :],
                        start=(kk == 0), stop=(kk == nkk - 1), perf_mode=SWI,
                    )

            def emit_h2(src):
                pm = psmall.tile([128, NT], f32, tag="pm", name="pm")
                for kk in range(KKZ):
                    nc.tensor.matmul(
                        pm[0:M2, :], hw2_sb[:, kk, :, :], src[:, kk, :, :],
                        start=(kk == 0), stop=(kk == KKZ - 1), perf_mode=DR,
                    )
                return pm

            def emit_chain(t, pm):
                """tanh/square/apre-mul — runs during the next pair's body."""
                mv = small.tile([TPART, NT], f32r, tag="mv", name="mv")
                nc.scalar.activation(
                    mv[:], pm[0:TPART, :], Act.Tanh,
                    bias=hb2c_sb[:, 0:1], scale=1.0 / P_SC,
                )
                sq = small.tile([TPART, NT], f32r, tag="sq", name="sq")
                nc.scalar.square(sq[:], mv[:])
                am = small.tile([TPART, NT], f32r, tag="am", name="am")
                nc.vector.tensor_mul(am[:], mv[:], apre_sb[t][:])
                return sq, am

            def emit_red(t, pm, sq, am):
                """reduce into row 0 of the (consumed) head-2 bank + store."""
                pl = pm[0:1, :]
                nc.tensor.matmul(pl, wredW_sb[:], sq[:],
                                 start=True, stop=False)
                nc.tensor.matmul(pl, onesW_sb[:], am[:],
                                 start=False, stop=True)
                o = small.tile([1, NT], f32, tag="o", name="o")
                nc.vector.tensor_copy(o[:], pl)
                nc.sync.dma_start(out[t : t + 1, :], o[:])

            loop_cm = tc.For_i(0, repeats, 1) if repeats > 1 else nullcontext()
            with loop_cm:
                pending = []
                for p in range(CT // 2):
                    tA, tB = 2 * p, 2 * p + 1

                    def lagged(nblk):
                        """(side, m) sequence with B one block behind A, so
                        each tile's last relu lands well before its next
                        layer's first matmul."""
                        seq = [("A", 0)]
                        for m in range(1, nblk):
                            seq += [("A", m), ("B", m - 1)]
                        seq.append(("B", nblk - 1))
                        return seq

                    h0A = acts.tile([128, KKH, 2, NT], fp8, tag="h0", name="h0")
                    h0B = acts.tile([128, KKH, 2, NT], fp8, tag="h0", name="h0")
                    for side, m in lagged(KH):
                        wblk = w0_sb[:, bass.ts(m, 128)]
                        src = xt_sb[tA] if side == "A" else xt_sb[tB]
                        dst = h0A if side == "A" else h0B
                        ps = psum.tile([128, NT], f32, tag="ps", name="ps")
                        nc.tensor.matmul(ps[:], wblk, src[:],
                                         start=True, stop=True)
                        relu_l0(dst, m, ps,
                                scalar_engine=((m + (side == "B")) % 2 == 0))

                    def dr_layer(w_sb, src_A, src_B, dst_A, dst_B,
                                 bias_sb, emap, nblk, nkk):
                        for side, m in lagged(nblk):
                            src = src_A if side == "A" else src_B
                            dst = dst_A if side == "A" else dst_B
                            ps = psum.tile([128, NT], f32, tag="ps",
                                           name="ps")
                            emit_dr_group(w_sb, src, ps, m, nkk)
                            emit_relu(dst, m, ps, bias_sb, emap)

                    h1A = acts.tile([128, KKH, 2, NT], fp8, tag="h1", name="h1")
                    h1B = acts.tile([128, KKH, 2, NT], fp8, tag="h1", name="h1")
                    dr_layer(w1_sb, h0A, h0B, h1A, h1B, b1_sb, MAP_L1, KH, KKH)

                    h2A = acts.tile([128, KKH, 2, NT], fp8, tag="h2", name="h2")
                    h2B = acts.tile([128, KKH, 2, NT], fp8, tag="h2", name="h2")
                    dr_layer(w2_sb, h1A, h1B, h2A, h2B, b2_sb, MAP_L2, KH, KKH)

                    # deferred reduces from the previous pair: their chains
                    # have been running since that pair's end, so the two
                    # small matmuls issue with inputs already in SBUF
                    while pending:
                        emit_red(*pending.pop(0))

                    zA = acts.tile([128, KKZ, 2, NT], fp8, tag="z", name="z")
                    zB = acts.tile([128, KKZ, 2, NT], fp8, tag="z", name="z")
                    dr_layer(hw1_sb, h2A, h2B, zA, zB, hb1_sb, MAP_H1, KZ, KKH)

                    pmA = emit_h2(zA)
                    pmB = emit_h2(zB)
                    sqA, amA = emit_chain(tA, pmA)
                    sqB, amB = emit_chain(tB, pmB)
                    pending.append((tA, pmA, sqA, amA))
                    pending.append((tB, pmB, sqB, amB))
                while pending:
                    emit_red(*pending.pop(0))

    nc.compile()
    return nc


def _get_graph(repeats=1):
    if repeats not in _GRAPHS:
        _GRAPHS[repeats] = _build_graph(repeats)
    return _GRAPHS[repeats]


def _round_f32r(a):
    """Round fp32 to the PE's fp32r format (11-bit mantissa, low 12 bits 0)."""
    b = np.ascontiguousarray(a, dtype=np.float32).view(np.uint32)
    lsb = (b >> np.uint32(12)) & np.uint32(1)
    out = (b + np.uint32(0x7FF) + lsb) & np.uint32(0xFFFFF000)
    return out.view(np.float32)


def _fp8(a):
    return np.ascontiguousarray(np.asarray(a, dtype=np.float32).astype(F8))


def _block_scales(emap):
    """Stored-activation scale per output block for a DR layer."""
    return np.array([ALPHA if a else P_SC for a in emap], np.float32)


def _quant_w_dr(W, s_in):
    """Plain DoubleRow layout [128, K//256, 2, M] (used for head-2, whose
    narrow M is rejected by the SWI ldweights active-cols ISA check)."""
    K, M = W.shape
    c = np.repeat(P_SC / s_in, 128)[:, None]
    Wq = _fp8(c * W)
    return np.ascontiguousarray(
        Wq.reshape(K // 256, 2, 128, M).transpose(2, 0, 1, 3)
    )


def _quant_w(W, s_in, blk=128):
    """W: [K, M] -> fp8(c_k * W), c_k = P / s_in[block(k)], in the
    DoubleRowSwInterleave layout [128, K//256, M//blk, 2*blk]: per output
    block, the two contraction halves interleaved per column with columns
    reversed (A_{blk-1} B_{blk-1} ... A_0 B_0)."""
    K, M = W.shape
    c = np.repeat(P_SC / s_in, 128)[:, None]
    Wq = _fp8(c * W)
    wdr = Wq.reshape(K // 256, 2, 128, M).transpose(2, 0, 1, 3)
    kkn = K // 256
    nblk = M // blk
    wb = wdr.reshape(128, kkn, 2, nblk, blk)[:, :, :, :, ::-1]
    return np.ascontiguousarray(
        wb.transpose(0, 1, 3, 4, 2).reshape(128, kkn, nblk, 2 * blk)
    )


def _bias_cols(b, emap):
    """[128, nblk] f32, column m pre-scaled for its engine path."""
    nblk = len(emap)
    cols = b.reshape(nblk, 128).T.astype(np.float32).copy()
    for m, is_act in enumerate(emap):
        cols[:, m] *= ALPHA if is_act else P_SC
    return np.ascontiguousarray(cols)


def kernel(
    obs, role_ids, actions,
    W0, b0, W1, b1, W2, b2,
    hW1, hb1, hW2, hb2, log_stds,
):
    from concourse.bass_utils import run_bass_kernel_spmd

    obs = np.asarray(obs, dtype=np.float32)
    role_ids = np.asarray(role_ids)
    actions = np.asarray(actions, dtype=np.float32)
    W0 = np.asarray(W0, dtype=np.float32)
    b0 = np.asarray(b0, dtype=np.float32)
    W1 = np.asarray(W1, dtype=np.float32)
    b1 = np.asarray(b1, dtype=np.float32)
    W2 = np.asarray(W2, dtype=np.float32)
    b2 = np.asarray(b2, dtype=np.float32)
    hW1 = np.asarray(hW1, dtype=np.float32)
    hb1 = np.asarray(hb1, dtype=np.float32)
    hW2 = np.asarray(hW2, dtype=np.float32)
    hb2 = np.asarray(hb2, dtype=np.float32)
    log_stds = np.asarray(log_stds, dtype=np.float32)

    nb, na = role_ids.shape
    obs_f = _round_f32r(obs.reshape(-1, OBS_DIM))
    act_f = actions.reshape(-1, ACTION_DIM)
    roles_f = role_ids.reshape(-1)
    n_tok = roles_f.shape[0]

    # ---- tile lists per role (token index + scatter destination) ----------
    tiles = []
    for r in range(NUM_ROLES):
        idx = np.nonzero(roles_f == r)[0]
        n = idx.shape[0]
        for s in range(0, n, NT):
            chunk = idx[s : s + NT]
            tok = np.zeros(NT, dtype=np.int64)
            dst = np.full(NT, -1, dtype=np.int64)
            tok[: chunk.shape[0]] = chunk
            dst[: chunk.shape[0]] = chunk
            tiles.append((r, tok, dst))

    out_full = np.zeros(n_tok, dtype=np.float32)
    if not tiles:
        return out_full.reshape(nb, na)

    # ---- static weight payloads -------------------------------------------
    s0 = np.ones(KH, np.float32)  # layer-0 blocks store 1*h on both engines
    s1 = _block_scales(MAP_L1)
    s2 = _block_scales(MAP_L2)
    s3 = _block_scales(MAP_H1)

    w0_dev = _round_f32r(W0[:OBS_DIM])
    w1_dev = _quant_w(W1, s0)
    w2_dev = _quant_w(W2, s1)
    b1_dev = _bias_cols(b1, MAP_L1)
    b2_dev = _bias_cols(b2, MAP_L2)

    ones_col = np.ones((TPART, 1), np.float32)

    role_payload = {}
    role_w = {}
    role_c = {}
    for r in range(NUM_ROLES):
        b0p = b0 + W0[OBS_DIM + r]
        inv_std = np.exp(-log_stds[r]).astype(np.float64)
        w_vec = (-0.5 * inv_std * inv_std).astype(np.float32)  # [8]
        c_v = np.float32(-np.sum(log_stds[r]) - 0.5 * LOG_2PI * ACTION_DIM)
        role_w[r] = w_vec
        role_c[r] = c_v
        hw2_pad = np.zeros((H2, M2), np.float32)
        hw2_pad[:, :ACTION_DIM] = hW2[r]
        wred_col = np.zeros((TPART, 1), np.float32)
        wred_col[:ACTION_DIM, 0] = w_vec
        hb2_col = np.zeros((TPART, 1), np.float32)
        hb2_col[:ACTION_DIM, 0] = hb2[r]
        hb2_col[ACTION_DIM, 0] = 20.0  # tanh(20) == 1.0 const lane
        b0_cols = np.ascontiguousarray(
            b0p.reshape(KH, 128).T.astype(np.float32)
        )
        role_payload[r] = dict(
            w0=w0_dev,
            bias0=b0_cols,
            w1=w1_dev, bias1=b1_dev,
            w2=w2_dev, bias2=b2_dev,
            hw1=_quant_w(hW1[r], s2),
            hbias1=_bias_cols(hb1[r], MAP_H1),
            hw2=_quant_w_dr(hw2_pad, s3),
            hb2c=hb2_col,
            wredW=_round_f32r(wred_col),
            onesW=_round_f32r(ones_col),
        )

    nc = _get_graph(int(getattr(kernel, "nrep", 1)))

    # ---- pack tiles into batches of N_CORES cores x CT single-role tiles --
    batches = []
    i = 0
    while i < len(tiles):
        cores = []
        for _ in range(N_CORES):
            if i >= len(tiles):
                cores.append((0, []))
                continue
            role = tiles[i][0]
            group = []
            while i < len(tiles) and tiles[i][0] == role and len(group) < CT:
                group.append(tiles[i][1:])
                i += 1
            cores.append((role, group))
        batches.append(cores)

    for cores in batches:
        in_maps = []
        scatter = []
        for ci, (role, group) in enumerate(cores):
            toks = [g[0] for g in group]
            dsts = [g[1] for g in group]
            while len(toks) < CT:
                toks.append(np.zeros(NT, dtype=np.int64))
                dsts.append(np.full(NT, -1, dtype=np.int64))
            tok_pad = np.concatenate(toks)
            dst_pad = np.concatenate(dsts)
            xT_c = np.ascontiguousarray(
                obs_f[tok_pad].T.reshape(128, CT, NT).transpose(1, 0, 2)
            )
            # tail payloads: apre rows 0..7 = -2*w_k*a_k, and the constant
            # row 8 = sum_k w_k a_k^2 + c (rides the tanh=1 lane)
            w_vec = role_w[role]
            c_v = role_c[role]
            a_all = act_f[tok_pad]  # [C, 8]
            apre_c = np.zeros((CT, TPART, NT), np.float32)
            for t in range(CT):
                a_t = a_all[t * NT : (t + 1) * NT]  # [NT, 8]
                apre_c[t, :ACTION_DIM, :] = -2.0 * w_vec[:, None] * a_t.T
                apre_c[t, ACTION_DIM, :] = (
                    (w_vec[None, :] * a_t * a_t).sum(1) + c_v
                )
            m = dict(role_payload[role])
            m["xT"] = xT_c
            m["apre"] = _round_f32r(apre_c)
            in_maps.append(m)
            scatter.append(dst_pad)

        res = run_bass_kernel_spmd(nc, in_maps, list(range(N_CORES)))
        for ci in range(N_CORES):
            vals = np.asarray(res.results[ci]["out"]).reshape(-1)
            dst = scatter[ci]
            valid = dst >= 0
            out_full[dst[valid]] = vals[valid]

    return out_full.reshape(nb, na)


# revision 30
# speedup vs baseline: 1.1734x; 1.1734x over previous
"""Trainium2 Bass kernel for nn_ActorMultiHead (moe_routing).

Strategy
--------
The reference runs every role head on every token (dense form of a masked
dispatch) and then selects the row matching the token's role; tokens whose
role >= NUM_ROLES contribute exactly 0.  We implement the sparse dispatch:

  * Host: flatten [B, A] tokens, drop role>=2 tokens (their output is 0),
    sort the rest by role, pack them into NT-token single-role tiles, and
    assign whole tiles to cores so every core serves exactly one role.
    The one-hot input contribution then folds into the layer-0 bias and the
    per-role heads become per-core weight *data* — the SPMD graph is
    role-agnostic.  NT=456 so 6 tiles x 8 cores exactly covers the ~21.8k
    active tokens (24 tiles per role -> 4 cores per role).
  * Device (per core, 3 pairs of tiles x NT tokens), mixed precision:
        layer 0 (obs->H)      : f32r matmul, contraction 128; both relu
                                paths store 1*h so blocks alternate engines
        layers 1,2 (H->H)     : fp8e4m3 DoubleRow matmuls (contraction 256)
        head 1 (H->H2)        : fp8 DoubleRow
        head 2 (H2->act)      : fp8 DoubleRow, M padded 8->16 (col 8 is the
                                const lane)
        log-prob tail         : lp = sum_k w_k (a_k - mean_k)^2 + c expanded
                                as W_red^T mean^2 + 1^T (mean * apre) with
                                apre = -2*w*a precomputed on host; the
                                constant ca = sum(w*a^2)+c rides in apre
                                row 8 against a tanh(bias=20)=1.0 lane.
                                The tanh/square/apre-mul chain is emitted at
                                the pair's end (runs during the next pair's
                                body); only the two small reduce matmuls +
                                copy + store are deferred to the next pair's
                                L2/H1 boundary, so the PE never waits.
    fp8 scaling: every fp8-layer PSUM uniformly holds P*u (P=16).  DR-layer
    ReLU blocks split between ScalarE (stores alpha*h, alpha=4) and VectorE
    (stores P*h); the per-block scale folds into the NEXT layer's fp8
    weight quantization (c_k = P / s_block(k)).  Layer-0 stores 1*h on both
    engines so each block's two tiles use both engines concurrently.
  * Host: scatter per-tile outputs back to original token positions.

For timing, a variant graph wraps the whole per-core compute in a
constant-bound For_i loop (set `kernel.nrep = R`), so a test harness can
measure HW time as (wall(nrep=R) - wall(nrep=1)) / (R-1).
"""

import math

import numpy as np
import ml_dtypes

# -- problem constants (from the problem statement, hardcoded) ---------------
B, A = 2048, 16
OBS_DIM, HIDDEN, ACTION_DIM = 128, 1024, 8
NUM_ROLES = 2
AGENT_ID_DIM = NUM_ROLES
H2 = HIDDEN // 2
LOG_2PI = math.log(2.0 * math.pi)

N_CORES = 8
NT = 456          # tokens per tile (matmul moving free dim)
CT = 6            # tiles per core (fixed compile-time shape)
C = NT * CT       # tokens per core per batch
KH = HIDDEN // 128    # 8 feature blocks of hidden
KZ = H2 // 128        # 4 feature blocks of the head hidden
KKH = KH // 2         # 4 DoubleRow k-pair tiles over hidden
KKZ = KZ // 2         # 2 DoubleRow k-pair tiles over head hidden
M2 = 16               # head-2 output padded to 16 (col 8 = const lane)

P_SC = 16.0       # uniform fp8-layer PSUM scale
ALPHA = 4.0       # ScalarE-block stored-activation scale (DR layers)

# per-block engine maps for DR layers: True -> ScalarE, False -> VectorE
MAP_L1 = (True, True, True, True, True, False, False, False)
MAP_L2 = (True, True, True, True, True, False, False, False)
MAP_H1 = (True, True, False, False)

F8 = ml_dtypes.float8_e4m3

_GRAPHS = {}  # repeats -> compiled graph, built once per process

TPART = 9  # tail partitions (rows 0..7 = actions, row 8 = const lane)


def _build_graph(repeats=1):
    import concourse.bass as bass
    import concourse.tile as tile
    from concourse import bacc, mybir

    f32 = mybir.dt.float32
    f32r = mybir.dt.float32r
    fp8 = mybir.dt.float8e4
    Act = mybir.ActivationFunctionType
    Alu = mybir.AluOpType
    DR = mybir.MatmulPerfMode.DoubleRow

    nc = bacc.Bacc(None, target_bir_lowering=False)

    xT = nc.declare_dram_parameter("xT", [CT, 128, NT], f32r, isOutput=False)
    w0 = nc.declare_dram_parameter("w0", [128, HIDDEN], f32r, isOutput=False)
    bias0 = nc.declare_dram_parameter("bias0", [128, KH], f32, isOutput=False)
    w1 = nc.declare_dram_parameter("w1", [128, KKH, 2, HIDDEN], fp8, isOutput=False)
    bias1 = nc.declare_dram_parameter("bias1", [128, KH], f32, isOutput=False)
    w2 = nc.declare_dram_parameter("w2", [128, KKH, 2, HIDDEN], fp8, isOutput=False)
    bias2 = nc.declare_dram_parameter("bias2", [128, KH], f32, isOutput=False)
    hw1 = nc.declare_dram_parameter("hw1", [128, KKH, 2, H2], fp8, isOutput=False)
    hbias1 = nc.declare_dram_parameter("hbias1", [128, KZ], f32, isOutput=False)
    hw2 = nc.declare_dram_parameter("hw2", [128, KKZ, 2, M2], fp8, isOutput=False)
    hb2c = nc.declare_dram_parameter("hb2c", [TPART, 1], f32, isOutput=False)
    wredW = nc.declare_dram_parameter("wredW", [TPART, 1], f32r, isOutput=False)
    onesW = nc.declare_dram_parameter("onesW", [TPART, 1], f32r, isOutput=False)
    apre = nc.declare_dram_parameter("apre", [CT, TPART, NT], f32r, isOutput=False)
    out = nc.declare_dram_parameter("out", [CT, NT], f32, isOutput=True)

    with tile.TileContext(nc) as tc:
        with (
            tc.tile_pool(name="consts", bufs=1) as consts,
            tc.tile_pool(name="acts", bufs=2) as acts,
            tc.tile_pool(name="small", bufs=2) as small,
            tc.tile_pool(name="psum", bufs=6, space="PSUM") as psum,
            tc.tile_pool(name="psmall", bufs=2, space="PSUM") as psmall,
        ):
            # resident inputs, DMA'd roughly in the order compute needs them
            xt_sb = []
            xt0 = consts.tile([128, NT], f32r, name="xt0", uniquify=True)
            nc.sync.dma_start(xt0[:], xT[0])
            xt1 = consts.tile([128, NT], f32r, name="xt1", uniquify=True)
            nc.sync.dma_start(xt1[:], xT[1])
            xt_sb += [xt0, xt1]
            w0_sb = consts.tile([128, HIDDEN], f32r)
            nc.sync.dma_start(w0_sb[:], w0[:])
            b0_sb = consts.tile([128, KH], f32)
            nc.sync.dma_start(b0_sb[:], bias0[:])
            for t in range(2, CT):
                xt = consts.tile([128, NT], f32r, name=f"xt{t}", uniquify=True)
                nc.sync.dma_start(xt[:], xT[t])
                xt_sb.append(xt)
            w1_sb = consts.tile([128, KKH, 2, HIDDEN], fp8)
            nc.sync.dma_start(w1_sb[:], w1[:])
            b1_sb = consts.tile([128, KH], f32)
            nc.sync.dma_start(b1_sb[:], bias1[:])
            w2_sb = consts.tile([128, KKH, 2, HIDDEN], fp8)
            nc.sync.dma_start(w2_sb[:], w2[:])
            b2_sb = consts.tile([128, KH], f32)
            nc.sync.dma_start(b2_sb[:], bias2[:])
            hw1_sb = consts.tile([128, KKH, 2, H2], fp8)
            nc.sync.dma_start(hw1_sb[:], hw1[:])
            hb1_sb = consts.tile([128, KZ], f32)
            nc.sync.dma_start(hb1_sb[:], hbias1[:])
            hw2_sb = consts.tile([128, KKZ, 2, M2], fp8)
            nc.sync.dma_start(hw2_sb[:], hw2[:])
            hb2c_sb = consts.tile([TPART, 1], f32)
            nc.sync.dma_start(hb2c_sb[:], hb2c[:])
            wredW_sb = consts.tile([TPART, 1], f32r)
            nc.sync.dma_start(wredW_sb[:], wredW[:])
            onesW_sb = consts.tile([TPART, 1], f32r)
            nc.sync.dma_start(onesW_sb[:], onesW[:])
            apre_sb = []
            for t in range(CT):
                ap_t = consts.tile([TPART, NT], f32r, name=f"apre{t}",
                                   uniquify=True)
                nc.sync.dma_start(ap_t[:], apre[t])
                apre_sb.append(ap_t)

            from contextlib import nullcontext

            def relu_l0(dest, m, ps, scalar_engine):
                # both engines store 1*h for layer 0
                dst = dest[:, m // 2, m % 2, :]
                bcol = b0_sb[:, m : m + 1]
                if scalar_engine:
                    nc.scalar.activation(dst, ps[:], Act.Relu, bias=bcol,
                                         scale=1.0)
                else:
                    nc.vector.tensor_scalar(dst, ps[:], bcol, 0.0,
                                            Alu.add, Alu.max)

            def emit_relu(dest, m, ps, bias_sb, emap):
                dst = dest[:, m // 2, m % 2, :]
                bcol = bias_sb[:, m : m + 1]
                if emap[m]:
                    nc.scalar.activation(dst, ps[:], Act.Relu, bias=bcol,
                                         scale=ALPHA / P_SC)
                else:
                    nc.vector.tensor_scalar(dst, ps[:], bcol, 0.0,
                                            Alu.add, Alu.max)

            def emit_dr_group(w_sb, src, ps, m, nkk):
                wm = w_sb[:, :, :, bass.ts(m, 128)]
                for kk in range(nkk):
                    nc.tensor.matmul(
                        ps[:], wm[:, kk, :, :], src[:, kk, :, :],
                        start=(kk == 0), stop=(kk == nkk - 1), perf_mode=DR,
                    )

            def emit_h2(src):
                pm = psmall.tile([128, NT], f32, tag="pm", name="pm")
                for kk in range(KKZ):
                    nc.tensor.matmul(
                        pm[0:M2, :], hw2_sb[:, kk, :, :], src[:, kk, :, :],
                        start=(kk == 0), stop=(kk == KKZ - 1), perf_mode=DR,
                    )
                return pm

            def emit_chain(t, pm):
                """tanh/square/apre-mul — runs during the next pair's body."""
                mv = small.tile([TPART, NT], f32r, tag="mv", name="mv")
                nc.scalar.activation(
                    mv[:], pm[0:TPART, :], Act.Tanh,
                    bias=hb2c_sb[:, 0:1], scale=1.0 / P_SC,
                )
                sq = small.tile([TPART, NT], f32r, tag="sq", name="sq")
                nc.scalar.square(sq[:], mv[:])
                am = small.tile([TPART, NT], f32r, tag="am", name="am")
                nc.vector.tensor_mul(am[:], mv[:], apre_sb[t][:])
                return sq, am

            def emit_red(t, pm, sq, am):
                """reduce into row 0 of the (consumed) head-2 bank + store."""
                pl = pm[0:1, :]
                nc.tensor.matmul(pl, wredW_sb[:], sq[:],
                                 start=True, stop=False)
                nc.tensor.matmul(pl, onesW_sb[:], am[:],
                                 start=False, stop=True)
                o = small.tile([1, NT], f32, tag="o", name="o")
                nc.vector.tensor_copy(o[:], pl)
                nc.sync.dma_start(out[t : t + 1, :], o[:])

            loop_cm = tc.For_i(0, repeats, 1) if repeats > 1 else nullcontext()
            with loop_cm:
                pending = []
                for p in range(CT // 2):
                    tA, tB = 2 * p, 2 * p + 1

                    def lagged(nblk):
                        """(side, m) sequence with B one block behind A, so
                        each tile's last relu lands well before its next
                        layer's first matmul."""
                        seq = [("A", 0)]
                        for m in range(1, nblk):
                            seq += [("A", m), ("B", m - 1)]
                        seq.append(("B", nblk - 1))
                        return seq

                    h0A = acts.tile([128, KKH, 2, NT], fp8, tag="h0", name="h0")
                    h0B = acts.tile([128, KKH, 2, NT], fp8, tag="h0", name="h0")
                    for side, m in lagged(KH):
                        wblk = w0_sb[:, bass.ts(m, 128)]
                        src = xt_sb[tA] if side == "A" else xt_sb[tB]
                        dst = h0A if side == "A" else h0B
                        ps = psum.tile([128, NT], f32, tag="ps", name="ps")
                        nc.tensor.matmul(ps[:], wblk, src[:],
                                         start=True, stop=True)
                        relu_l0(dst, m, ps,
                                scalar_engine=((m + (side == "B")) % 2 == 0))

                    def dr_layer(w_sb, src_A, src_B, dst_A, dst_B,
                                 bias_sb, emap, nblk, nkk):
                        for side, m in lagged(nblk):
                            src = src_A if side == "A" else src_B
                            dst = dst_A if side == "A" else dst_B
                            ps = psum.tile([128, NT], f32, tag="ps",
                                           name="ps")
                            emit_dr_group(w_sb, src, ps, m, nkk)
                            emit_relu(dst, m, ps, bias_sb, emap)

                    h1A = acts.tile([128, KKH, 2, NT], fp8, tag="h1", name="h1")
                    h1B = acts.tile([128, KKH, 2, NT], fp8, tag="h1", name="h1")
                    dr_layer(w1_sb, h0A, h0B, h1A, h1B, b1_sb, MAP_L1, KH, KKH)

                    h2A = acts.tile([128, KKH, 2, NT], fp8, tag="h2", name="h2")
                    h2B = acts.tile([128, KKH, 2, NT], fp8, tag="h2", name="h2")
                    dr_layer(w2_sb, h1A, h1B, h2A, h2B, b2_sb, MAP_L2, KH, KKH)

                    # deferred reduces from the previous pair: their chains
                    # have been running since that pair's end, so the two
                    # small matmuls issue with inputs already in SBUF
                    while pending:
                        emit_red(*pending.pop(0))

                    zA = acts.tile([128, KKZ, 2, NT], fp8, tag="z", name="z")
                    zB = acts.tile([128, KKZ, 2, NT], fp8, tag="z", name="z")
                    dr_layer(hw1_sb, h2A, h2B, zA, zB, hb1_sb, MAP_H1, KZ, KKH)

                    pmA = emit_h2(zA)
                    pmB = emit_h2(zB)
                    sqA, amA = emit_chain(tA, pmA)
                    sqB, amB = emit_chain(tB, pmB)
                    pending.append((tA, pmA, sqA, amA))
                    pending.append((tB, pmB, sqB, amB))
                while pending:
                    emit_red(*pending.pop(0))

    nc.compile()
    return nc


def _get_graph(repeats=1):
    if repeats not in _GRAPHS:
        _GRAPHS[repeats] = _build_graph(repeats)
    return _GRAPHS[repeats]


def _round_f32r(a):
    """Round fp32 to the PE's fp32r format (11-bit mantissa, low 12 bits 0)."""
    b = np.ascontiguousarray(a, dtype=np.float32).view(np.uint32)
    lsb = (b >> np.uint32(12)) & np.uint32(1)
    out = (b + np.uint32(0x7FF) + lsb) & np.uint32(0xFFFFF000)
    return out.view(np.float32)


def _fp8(a):
    return np.ascontiguousarray(np.asarray(a, dtype=np.float32).astype(F8))


def _block_scales(emap):
    """Stored-activation scale per output block for a DR layer."""
    return np.array([ALPHA if a else P_SC for a in emap], np.float32)


def _quant_w(W, s_in):
    """W: [K, M] -> fp8(c_k * W), c_k = P / s_in[block(k)], in DoubleRow
    layout [128, K//256, 2, M]."""
    K, M = W.shape
    c = np.repeat(P_SC / s_in, 128)[:, None]
    Wq = _fp8(c * W)
    return np.ascontiguousarray(
        Wq.reshape(K // 256, 2, 128, M).transpose(2, 0, 1, 3)
    )


def _bias_cols(b, emap):
    """[128, nblk] f32, column m pre-scaled for its engine path."""
    nblk = len(emap)
    cols = b.reshape(nblk, 128).T.astype(np.float32).copy()
    for m, is_act in enumerate(emap):
        cols[:, m] *= ALPHA if is_act else P_SC
    return np.ascontiguousarray(cols)


def kernel(
    obs, role_ids, actions,
    W0, b0, W1, b1, W2, b2,
    hW1, hb1, hW2, hb2, log_stds,
):
    from concourse.bass_utils import run_bass_kernel_spmd

    obs = np.asarray(obs, dtype=np.float32)
    role_ids = np.asarray(role_ids)
    actions = np.asarray(actions, dtype=np.float32)
    W0 = np.asarray(W0, dtype=np.float32)
    b0 = np.asarray(b0, dtype=np.float32)
    W1 = np.asarray(W1, dtype=np.float32)
    b1 = np.asarray(b1, dtype=np.float32)
    W2 = np.asarray(W2, dtype=np.float32)
    b2 = np.asarray(b2, dtype=np.float32)
    hW1 = np.asarray(hW1, dtype=np.float32)
    hb1 = np.asarray(hb1, dtype=np.float32)
    hW2 = np.asarray(hW2, dtype=np.float32)
    hb2 = np.asarray(hb2, dtype=np.float32)
    log_stds = np.asarray(log_stds, dtype=np.float32)

    nb, na = role_ids.shape
    obs_f = _round_f32r(obs.reshape(-1, OBS_DIM))
    act_f = actions.reshape(-1, ACTION_DIM)
    roles_f = role_ids.reshape(-1)
    n_tok = roles_f.shape[0]

    # ---- tile lists per role (token index + scatter destination) ----------
    tiles = []
    for r in range(NUM_ROLES):
        idx = np.nonzero(roles_f == r)[0]
        n = idx.shape[0]
        for s in range(0, n, NT):
            chunk = idx[s : s + NT]
            tok = np.zeros(NT, dtype=np.int64)
            dst = np.full(NT, -1, dtype=np.int64)
            tok[: chunk.shape[0]] = chunk
            dst[: chunk.shape[0]] = chunk
            tiles.append((r, tok, dst))

    out_full = np.zeros(n_tok, dtype=np.float32)
    if not tiles:
        return out_full.reshape(nb, na)

    # ---- static weight payloads -------------------------------------------
    s0 = np.ones(KH, np.float32)  # layer-0 blocks store 1*h on both engines
    s1 = _block_scales(MAP_L1)
    s2 = _block_scales(MAP_L2)
    s3 = _block_scales(MAP_H1)

    w0_dev = _round_f32r(W0[:OBS_DIM])
    w1_dev = _quant_w(W1, s0)
    w2_dev = _quant_w(W2, s1)
    b1_dev = _bias_cols(b1, MAP_L1)
    b2_dev = _bias_cols(b2, MAP_L2)

    ones_col = np.ones((TPART, 1), np.float32)

    role_payload = {}
    role_w = {}
    role_c = {}
    for r in range(NUM_ROLES):
        b0p = b0 + W0[OBS_DIM + r]
        inv_std = np.exp(-log_stds[r]).astype(np.float64)
        w_vec = (-0.5 * inv_std * inv_std).astype(np.float32)  # [8]
        c_v = np.float32(-np.sum(log_stds[r]) - 0.5 * LOG_2PI * ACTION_DIM)
        role_w[r] = w_vec
        role_c[r] = c_v
        hw2_pad = np.zeros((H2, M2), np.float32)
        hw2_pad[:, :ACTION_DIM] = hW2[r]
        wred_col = np.zeros((TPART, 1), np.float32)
        wred_col[:ACTION_DIM, 0] = w_vec
        hb2_col = np.zeros((TPART, 1), np.float32)
        hb2_col[:ACTION_DIM, 0] = hb2[r]
        hb2_col[ACTION_DIM, 0] = 20.0  # tanh(20) == 1.0 const lane
        b0_cols = np.ascontiguousarray(
            b0p.reshape(KH, 128).T.astype(np.float32)
        )
        role_payload[r] = dict(
            w0=w0_dev,
            bias0=b0_cols,
            w1=w1_dev, bias1=b1_dev,
            w2=w2_dev, bias2=b2_dev,
            hw1=_quant_w(hW1[r], s2),
            hbias1=_bias_cols(hb1[r], MAP_H1),
            hw2=_quant_w(hw2_pad, s3),
            hb2c=hb2_col,
            wredW=_round_f32r(wred_col),
            onesW=_round_f32r(ones_col),
        )

    nc = _get_graph(int(getattr(kernel, "nrep", 1)))

    # ---- pack tiles into batches of N_CORES cores x CT single-role tiles --
    batches = []
    i = 0
    while i < len(tiles):
        cores = []
        for _ in range(N_CORES):
            if i >= len(tiles):
                cores.append((0, []))
                continue
            role = tiles[i][0]
            group = []
            while i < len(tiles) and tiles[i][0] == role and len(group) < CT:
                group.append(tiles[i][1:])
                i += 1
            cores.append((role, group))
        batches.append(cores)

    for cores in batches:
        in_maps = []
        scatter = []
        for ci, (role, group) in enumerate(cores):
            toks = [g[0] for g in group]
            dsts = [g[1] for g in group]
            while len(toks) < CT:
                toks.append(np.zeros(NT, dtype=np.int64))
                dsts.append(np.full(NT, -1, dtype=np.int64))
            tok_pad = np.concatenate(toks)
            dst_pad = np.concatenate(dsts)
            xT_c = np.ascontiguousarray(
                obs_f[tok_pad].T.reshape(128, CT, NT).transpose(1, 0, 2)
            )
            # tail payloads: apre rows 0..7 = -2*w_k*a_k, and the constant
            # row 8 = sum_k w_k a_k^2 + c (rides the tanh=1 lane)
            w_vec = role_w[role]
            c_v = role_c[role]
            a_all = act_f[tok_pad]  # [C, 8]
            apre_c = np.zeros((CT, TPART, NT), np.float32)
            for t in range(CT):
                a_t = a_all[t * NT : (t + 1) * NT]  # [NT, 8]
                apre_c[t, :ACTION_DIM, :] = -2.0 * w_vec[:, None] * a_t.T
                apre_c[t, ACTION_DIM, :] = (
                    (w_vec[None, :] * a_t * a_t).sum(1) + c_v
                )
            m = dict(role_payload[role])
            m["xT"] = xT_c
            m["apre"] = _round_f32r(apre_c)
            in_maps.append(m)
            scatter.append(dst_pad)

        res = run_bass_kernel_spmd(nc, in_maps, list(range(N_CORES)))
        for ci in range(N_CORES):
            vals = np.asarray(res.results[ci]["out"]).reshape(-1)
            dst = scatter[ci]
            valid = dst >= 0
            out_full[dst[valid]] = vals[valid]

    return out_full.reshape(nb, na)
